# revision 40
# baseline (speedup 1.0000x reference)
"""MAGC (multi-header attention global context) pooling kernel for Trainium2.

Math (per sample, reference.py):
    xh[g, n, :]   = x[n, g*64:(g+1)*64]                (g=8 headers, n=H*W)
    logits[g, n]  = (xh[g, n, :] . w_mask + b_mask) / 8
    attn          = softmax_n(logits)
    ctx[g, :]     = sum_n attn[g, n] * xh[g, n, :]     -> ctx [C]
    t             = relu(LN(ctx @ w1 + b1)) @ w2 + b2
    out           = x + t  (broadcast over n)

Sharding: pure data parallel, 16 samples -> 8 cores x 2 samples.

v2 design (per core, 2 samples):
  - x is kept in SBUF as f16 ONLY (xh, 60KB/partition/sample); the f32
    stream passes through a small staging pool and is converted by ACT.
    Both samples' xh fit resident -> full cross-sample pipelining.
  - DMA layout "(p k) c": each partition holds 4 consecutive HBM rows =
    one contiguous 8KB line (4x fewer descriptors than "(k p) c").
    Softmax/ctx are permutation-invariant over positions so relabeling
    is free; the store uses the same relabeling.
  - in-DMAs issue from the sync (SP) HWDGE queue, out-DMAs from the ACT
    HWDGE queue so the two streams' triggers don't head-of-line block.
  - residual add runs on gpsimd (f16+f16 -> f32 out staging); DVE does
    the logits mul+segmented reduce; ACT does converts+exp; PE does ctx.
    DVE(TT) and gpsimd both lock the shared SBUF port pair, but their
    combined locked time per chunk (~2.8us) fits in the DMA slot.
  - emission: A0 | B0 | interleave(C0, A1) | B1 | C1, where A=load+attn,
    B=softmax+MLP (serial), C=add+store. C0/A1 interleave chunk-wise so
    the out and in DMA streams share HBM bandwidth.
"""

import sys

import numpy as np

if "/opt/trn_rl_repo" not in sys.path:
    sys.path.insert(0, "/opt/trn_rl_repo")

B, H, W, C = 16, 48, 160, 512
G = 8                 # attention headers
SHI = C // G          # 64 channels per header
N = H * W             # 7680 spatial positions per sample
P = 128               # SBUF partitions
NT = N // P           # 60 [128, C] tiles per sample
NCORES = 8
BPC = B // NCORES     # samples per core
NB = C // P           # 4 channel blocks of 128
LN_EPS = 1e-3
KCH = 4               # [128, C] tiles per processing chunk (1 MB DMAs)
NCHK = NT // KCH      # 15 chunks per sample
OUT_DMA_ON_ACT = False  # issue store DMAs from the ACT HWDGE queue
PK_LAYOUT = True      # contiguous 8KB-per-partition DMA lines ("(p k) c")
INTERLEAVE = False     # interleave C0 with A1
REDUCE_ON_GP = False   # segmented logits reduce on gpsimd (frees DVE in A)
ADD_ON_DVE = True     # residual add on DVE (frees gpsimd in C)
CAST_DMA_OUT = False  # store via gpsimd casting DMA (f16->f32), no staging
PE_ADD = False        # residual add on PE via ident/ones matmuls into PSUM
PREFETCH = 3          # sample-1 chunks loaded before sample-0 MLP/store
EXP_LAG = True       # delay exp/ctx one chunk behind the load stream
MUL_ON_GP = False      # logits multiply on gpsimd (frees DVE in A)


def build_nc():
    import concourse.tile as tile
    from concourse import bacc, mybir
    from concourse.bass import broadcast_tensor_aps

    f32 = mybir.dt.float32
    f16 = mybir.dt.float16
    AX = mybir.AxisListType.X
    MUL = mybir.AluOpType.mult
    ADD = mybir.AluOpType.add
    SUB = mybir.AluOpType.subtract
    AF = mybir.ActivationFunctionType

    nc = bacc.Bacc()

    x_d = nc.dram_tensor("x", [BPC, H, W, C], f32, kind="ExternalInput")
    wrep_d = nc.dram_tensor("w_rep", [P, KCH, C], f16, kind="ExternalInput")
    bb_d = nc.dram_tensor("b_bias", [P, 1], f32, kind="ExternalInput")
    w1_d = nc.dram_tensor("w1p", [P, NB, C], f16, kind="ExternalInput")
    w2_d = nc.dram_tensor("w2p", [P, NB, C], f16, kind="ExternalInput")
    b1_d = nc.dram_tensor("b1r", [1, C], f16, kind="ExternalInput")
    b2_d = nc.dram_tensor("b2r", [1, C], f16, kind="ExternalInput")
    gm_d = nc.dram_tensor("gammar", [1, C], f16, kind="ExternalInput")
    bt_d = nc.dram_tensor("betar", [1, C], f16, kind="ExternalInput")
    ms_d = nc.dram_tensor("mask_sel", [G, C], f16, kind="ExternalInput")
    id_d = nc.dram_tensor("ident8", [G, G], f32, kind="ExternalInput")
    oh_d = nc.dram_tensor("ones_h", [1, P], f16, kind="ExternalInput")
    idh_d = nc.dram_tensor("identh", [P, P], f16, kind="ExternalInput")
    oc_d = nc.dram_tensor("ones_c", [P, 1], f32, kind="ExternalInput")
    out_d = nc.dram_tensor("out", [BPC, H, W, C], f32, kind="ExternalOutput")

    xf = x_d.rearrange("b h w c -> (b h w) c")
    of = out_d.rearrange("b h w c -> (b h w) c")

    with tile.TileContext(nc) as tc:
        with (
            tc.tile_pool(name="consts", bufs=1) as consts,
            tc.tile_pool(name="xhp", bufs=2) as xhp,
            tc.tile_pool(name="esbp", bufs=2) as esbp,
            tc.tile_pool(name="xinp", bufs=3) as xinp,
            tc.tile_pool(name="xwp", bufs=1) as xwp,
            tc.tile_pool(name="lgp", bufs=2) as lgp,
            tc.tile_pool(name="xoutp", bufs=3) as xoutp,
            tc.tile_pool(name="trp", bufs=1) as trp,
            tc.tile_pool(name="smp", bufs=1) as smp,
            tc.tile_pool(name="ctxps", bufs=1, space="PSUM") as ctxps,
            tc.tile_pool(name="mps", bufs=2, space="PSUM") as mps,
            tc.tile_pool(name="tps", bufs=1, space="PSUM") as tps,
            tc.tile_pool(name="addps", bufs=4, space="PSUM") as addps,
        ):
            w_rep = consts.tile([P, KCH, C], f16)
            nc.sync.dma_start(w_rep, wrep_d[:, :, :])
            bb = consts.tile([P, 1], f32)
            nc.sync.dma_start(bb, bb_d[:, :])
            w1s = consts.tile([P, NB, C], f16)
            nc.sync.dma_start(w1s, w1_d[:, :, :])
            w2s = consts.tile([P, NB, C], f16)
            nc.sync.dma_start(w2s, w2_d[:, :, :])
            b1s = consts.tile([1, C], f16)
            nc.sync.dma_start(b1s, b1_d[:, :])
            b2s = consts.tile([1, C], f16)
            nc.sync.dma_start(b2s, b2_d[:, :])
            gms = consts.tile([1, C], f16)
            nc.sync.dma_start(gms, gm_d[:, :])
            bts = consts.tile([1, C], f16)
            nc.sync.dma_start(bts, bt_d[:, :])
            msel = consts.tile([G, C], f16)
            nc.sync.dma_start(msel, ms_d[:, :])
            ident8 = consts.tile([G, G], f32)
            nc.sync.dma_start(ident8, id_d[:, :])
            ones_h = consts.tile([1, P], f16)
            nc.sync.dma_start(ones_h, oh_d[:, :])
            identh = consts.tile([P, P], f16)
            nc.sync.dma_start(identh, idh_d[:, :])
            ones_c = consts.tile([P, 1], f32)
            nc.sync.dma_start(ones_c, oc_d[:, :])
            eps_t = consts.tile([1, 1], f32)
            nc.vector.memset(eps_t, LN_EPS)

            # Sem-absorption ops: walrus allows very few sync waits per
            # compute instruction, so let each engine observe the const-load
            # DMA sems via tiny reads up front, keeping hot-loop
            # instructions at <=1 wait each.
            ab_gp = smp.tile([1, 1], f32, tag="ab_gp")
            nc.gpsimd.tensor_copy(ab_gp, w_rep[0:1, 0, 0:1])
            ab_ac = smp.tile([1, 1], f32, tag="ab_ac")
            nc.scalar.copy(ab_ac, bb[0:1, 0:1])
            nc.scalar.copy(ab_ac, eps_t[0:1, 0:1])
            ab_dv = smp.tile([1, 1], f32, tag="ab_dv")
            nc.vector.tensor_copy(ab_dv, msel[0:1, 0:1])

            def phase_a_load(s, ck, xh):
                """Load chunk ck of sample s, convert, mul, segmented reduce.

                Returns the logits tile; exp/ctx are emitted one chunk later
                (phase_a_tail) so the ACT queue's exp never stalls the next
                chunk's convert behind the DVE reduce."""
                base = s * N
                t0 = ck * KCH
                xin = xinp.tile([P, KCH, C], f32, tag="xin")
                rows = xf[base + t0 * P : base + (t0 + KCH) * P, :]
                nc.sync.dma_start(xin, rows.rearrange(IN_PAT, **IN_KW))
                nc.scalar.copy(xh[:, t0 : t0 + KCH, :], xin)
                xw = xwp.tile([P, KCH, C], f16, tag="xw")
                MUL_ENGINE.tensor_mul(xw, xh[:, t0 : t0 + KCH, :], w_rep)
                lg = lgp.tile([P, KCH, G], f16, tag="lg")
                with nc.allow_low_precision(
                    reason="64-term f16 logit sums; |logits|<1, exp next"
                ):
                    REDUCE_ENGINE.reduce_sum(
                        lg, xw.rearrange("p k (g s) -> p k g s", s=SHI), AX
                    )
                return lg

            def phase_a_tail(s, ck, lg, xh, esb, ctx_ps):
                t0 = ck * KCH
                # E = exp((dot + b_mask) / 8); |logits| < ~1 so no
                # max-subtraction is needed for stability.
                nc.scalar.activation(
                    esb[:, t0 : t0 + KCH, :], lg, AF.Exp,
                    bias=bb[:, 0:1], scale=0.125,
                )
                for t in range(t0, t0 + KCH):
                    # fp16 single-pass PE matmul: ctx[g, c] += sum_p E * xh
                    nc.tensor.matmul(
                        ctx_ps,
                        esb[:, t, :],
                        xh[:, t, :],
                        start=(t == 0),
                        stop=(t == NT - 1),
                    )

            def phase_a_chunk(s, ck, xh, esb, ctx_ps, pend):
                lg = phase_a_load(s, ck, xh)
                pend.append((ck, lg))
                if len(pend) > (1 if EXP_LAG else 0):
                    cka, lga = pend.pop(0)
                    phase_a_tail(s, cka, lga, xh, esb, ctx_ps)

            def phase_a_flush(s, xh, esb, ctx_ps, pend):
                while pend:
                    cka, lga = pend.pop(0)
                    phase_a_tail(s, cka, lga, xh, esb, ctx_ps)

            def phase_b(s, esb, ctx_ps):
                """Softmax denominator + MLP; returns trep [P, KCH, C] f16."""
                # S[g] = sum_{p,t} E[p, t, g]
                sp = lgp.tile([P, G], f32, tag="lg")
                nc.vector.reduce_sum(sp, esb.rearrange("p t g -> p g t"), AX)
                s_ps = mps.tile([G, 1], f32, tag="m")
                nc.tensor.matmul(s_ps, sp, ones_c, start=True, stop=True)
                sinv = smp.tile([G, 1], f32, tag="sinv")
                nc.vector.reciprocal(sinv, s_ps)

                # ctx extract: scale rows by 1/S, mask to the diagonal
                # header blocks, transpose to channel-major [128, 4]
                ctx_sm = smp.tile([G, C], f32, tag="ctx_sm")
                nc.vector.scalar_tensor_tensor(
                    out=ctx_sm, in0=ctx_ps, scalar=sinv, in1=msel,
                    op0=MUL, op1=MUL,
                )
                tp_list = []
                for j in range(NB):
                    tp = mps.tile([P, G], f32, tag="m")
                    nc.tensor.transpose(
                        tp, ctx_sm[:, j * P : (j + 1) * P], ident8
                    )
                    tp_list.append(tp)
                ctxt = smp.tile([P, NB], f16, tag="ctxt")
                with nc.allow_low_precision(
                    reason="8-term masked sum; f16 ctx feeds f16 matmul"
                ):
                    for j in range(NB):
                        nc.vector.reduce_sum(
                            ctxt[:, j : j + 1], tp_list[j], AX
                        )

                # h = ctx @ w1 + b1
                h_ps = mps.tile([1, C], f32, tag="m")
                for j in range(NB):
                    nc.tensor.matmul(
                        h_ps, ctxt[:, j : j + 1], w1s[:, j, :],
                        start=(j == 0), stop=False,
                    )
                nc.tensor.matmul(
                    h_ps, ones_h[:, 0:1], b1s, start=False, stop=True
                )

                # LayerNorm over C, then ReLU (all on DVE except one sqrt)
                musum = smp.tile([1, 1], f32, tag="musum")
                nc.vector.reduce_sum(musum, h_ps, AX)
                mu = smp.tile([1, 1], f32, tag="mu")
                nc.vector.tensor_scalar_mul(mu, musum, 1.0 / C)
                hc = smp.tile([1, C], f32, tag="hc")
                nc.vector.tensor_scalar(
                    out=hc, in0=h_ps, scalar1=mu, scalar2=None, op0=SUB
                )
                sq = smp.tile([1, C], f32, tag="sq")
                varsum = smp.tile([1, 1], f32, tag="varsum")
                nc.scalar.activation(sq, hc, AF.Square, accum_out=varsum)
                std = smp.tile([1, 1], f32, tag="std")
                nc.scalar.activation(
                    std, varsum, AF.Sqrt, bias=eps_t[:, 0:1], scale=1.0 / C
                )
                rstd = smp.tile([1, 1], f32, tag="rstd")
                nc.vector.reciprocal(rstd, std)
                hn = smp.tile([1, C], f32, tag="sq")
                nc.vector.scalar_tensor_tensor(
                    out=hn, in0=hc, scalar=rstd, in1=gms, op0=MUL, op1=MUL
                )
                hb = smp.tile([1, C], f32, tag="hc")
                nc.vector.tensor_add(hb, hn, bts)
                rl = smp.tile([1, C], f32, tag="rl")
                nc.vector.tensor_scalar_max(rl, hb, 0.0)

                # t = relu_h @ w2 + b2 (transpose relu_h to [128, 4] first)
                rt_ps = mps.tile([P, NB], f32, tag="m")
                for j in range(NB):
                    nc.tensor.transpose(
                        rt_ps[:, j : j + 1],
                        rl[:, j * P : (j + 1) * P],
                        ones_c[0:1, 0:1],
                    )
                rts = smp.tile([P, NB], f16, tag="rts")
                nc.vector.tensor_copy(rts, rt_ps)
                t_ps = mps.tile([1, C], f32, tag="m")
                for j in range(NB):
                    nc.tensor.matmul(
                        t_ps, rts[:, j : j + 1], w2s[:, j, :],
                        start=(j == 0), stop=False,
                    )
                nc.tensor.matmul(
                    t_ps, ones_h[:, 0:1], b2s, start=False, stop=True
                )
                tsb = smp.tile([1, C], f16, tag="tsb")
                nc.vector.tensor_copy(tsb, t_ps)

                if PE_ADD:
                    return tsb
                trep_ps = tps.tile([P, C], f32, tag="trep")
                nc.tensor.matmul(trep_ps, ones_h, tsb, start=True, stop=True)
                trep = trp.tile([P, KCH, C], f16, tag="trep_sb")
                for k in range(KCH):
                    nc.vector.tensor_copy(trep[:, k, :], trep_ps)
                return trep

            OUT_DMA_ENGINE = (
                nc.scalar.dma_start if OUT_DMA_ON_ACT else nc.sync.dma_start
            )
            REDUCE_ENGINE = nc.gpsimd if REDUCE_ON_GP else nc.vector
            MUL_ENGINE = nc.gpsimd if MUL_ON_GP else nc.vector
            ADD_ENGINE = nc.vector if ADD_ON_DVE else nc.gpsimd
            if PK_LAYOUT:
                IN_PAT, IN_KW = "(p k) c -> p k c", {"p": P}
            else:
                IN_PAT, IN_KW = "(k p) c -> p k c", {"p": P}

            def phase_c_chunk(s, ck, xh, trep):
                """Residual add (gpsimd) + store (ACT HWDGE queue)."""
                base = s * N
                t0 = ck * KCH
                if PE_ADD:
                    for h in range(2):
                        tt = t0 + 2 * h
                        add_ps = addps.tile([P, 2 * C], f32, tag="add")
                        nc.tensor.matmul(
                            add_ps,
                            identh,
                            xh[:, tt : tt + 2, :].reshape(P, 2 * C),
                            start=True, stop=False,
                        )
                        nc.tensor.matmul(
                            add_ps, ones_h, trep, start=False, stop=True
                        )
                        xout = xoutp.tile([P, 2 * C], f32, tag="xout")
                        nc.scalar.copy(xout, add_ps)
                        rows = of[
                            base + tt * P : base + (tt + 2) * P, :
                        ]
                        OUT_DMA_ENGINE(
                            rows.rearrange(IN_PAT, **IN_KW).reshape(
                                P, 2 * C
                            ),
                            xout,
                        )
                    return
                rows = of[base + t0 * P : base + (t0 + KCH) * P, :]
                if CAST_DMA_OUT:
                    xadd = xwp.tile([P, KCH, C], f16, tag="xw")
                    with nc.allow_low_precision(
                        reason="residual add in f16; |out|<8, gate 2e-2"
                    ):
                        ADD_ENGINE.tensor_add(
                            xadd, xh[:, t0 : t0 + KCH, :], trep
                        )
                    # SWDGE casting DMA: f16 SBUF -> f32 HBM in flight
                    nc.gpsimd.dma_start(
                        rows.rearrange(IN_PAT, **IN_KW), xadd
                    )
                else:
                    xout = xoutp.tile([P, KCH, C], f32, tag="xout")
                    ADD_ENGINE.tensor_add(
                        xout, xh[:, t0 : t0 + KCH, :], trep
                    )
                    OUT_DMA_ENGINE(rows.rearrange(IN_PAT, **IN_KW), xout)

            # ---- sample 0: load + attention
            xh0 = xhp.tile([P, NT, C], f16, tag="xh")
            esb0 = esbp.tile([P, NT, G], f16, tag="esb")
            ctx0 = ctxps.tile([G, C], f32, tag="ctx")
            pend0 = []
            for ck in range(NCHK):
                phase_a_chunk(0, ck, xh0, esb0, ctx0, pend0)
            phase_a_flush(0, xh0, esb0, ctx0, pend0)
            # ---- prefetch first sample-1 chunks (their DMAs overlap B0)
            xh1 = xhp.tile([P, NT, C], f16, tag="xh")
            esb1 = esbp.tile([P, NT, G], f16, tag="esb")
            ctx1 = ctxps.tile([G, C], f32, tag="ctx")
            pend1 = []
            for ck in range(PREFETCH):
                phase_a_chunk(1, ck, xh1, esb1, ctx1, pend1)
            # ---- sample 0 MLP
            trep0 = phase_b(0, esb0, ctx0)
            # ---- interleave: store sample 0 / load+attn sample 1
            if INTERLEAVE:
                for ck in range(NCHK):
                    phase_c_chunk(0, ck, xh0, trep0)
                    if PREFETCH + ck < NCHK:
                        phase_a_chunk(
                            1, PREFETCH + ck, xh1, esb1, ctx1, pend1
                        )
            else:
                for ck in range(PREFETCH, NCHK):
                    phase_a_chunk(1, ck, xh1, esb1, ctx1, pend1)
                for ck in range(NCHK):
                    phase_c_chunk(0, ck, xh0, trep0)
            phase_a_flush(1, xh1, esb1, ctx1, pend1)
            # ---- sample 1 MLP + store
            trep1 = phase_b(1, esb1, ctx1)
            for ck in range(NCHK):
                phase_c_chunk(1, ck, xh1, trep1)

    nc.finalize()
    return nc


def _prep_shared(inputs):
    w_mask = np.asarray(inputs["w_mask"], np.float32).reshape(SHI)
    b_mask = np.asarray(inputs["b_mask"], np.float32).reshape(1)
    w1 = np.asarray(inputs["w1"], np.float32)
    w2 = np.asarray(inputs["w2"], np.float32)

    shared = {
        "w_rep": np.broadcast_to(
            np.tile(w_mask, G), (P, KCH, C)
        ).astype(np.float16),
        "b_bias": np.full((P, 1), b_mask[0] * 0.125, np.float32),
        "w1p": np.ascontiguousarray(
            w1.reshape(NB, P, C).transpose(1, 0, 2)
        ).astype(np.float16),
        "w2p": np.ascontiguousarray(
            w2.reshape(NB, P, C).transpose(1, 0, 2)
        ).astype(np.float16),
        "b1r": np.asarray(inputs["b1"], np.float16).reshape(1, C),
        "b2r": np.asarray(inputs["b2"], np.float16).reshape(1, C),
        "gammar": np.asarray(inputs["gamma"], np.float16).reshape(1, C),
        "betar": np.asarray(inputs["beta"], np.float16).reshape(1, C),
        "mask_sel": (
            (np.arange(C)[None, :] // SHI) == np.arange(G)[:, None]
        ).astype(np.float16),
        "ident8": np.eye(G, dtype=np.float32),
        "ones_h": np.ones((1, P), np.float16),
        "identh": np.eye(P, dtype=np.float16),
        "ones_c": np.ones((P, 1), np.float32),
    }
    return shared


def make_in_maps(inputs):
    x = np.asarray(inputs["x"], np.float32)
    shared = _prep_shared(inputs)
    in_maps = []
    for i in range(NCORES):
        m = dict(shared)
        m["x"] = np.ascontiguousarray(x[i * BPC : (i + 1) * BPC])
        in_maps.append(m)
    return in_maps


def _axon_device_reset():
    """Clear any wedged NRT exec-unit state left by a previous session."""
    try:
        import ctypes

        import jax

        jax.devices()
        lib = ctypes.CDLL("/opt/axon/libaxon_pjrt.so")
        lib.axon_reset.restype = ctypes.c_int64
        lib.axon_reset()
    except Exception:
        pass


def kernel(**inputs):
    from concourse.bass_utils import run_bass_kernel_spmd

    _axon_device_reset()
    nc = build_nc()
    in_maps = make_in_maps(inputs)
    res = run_bass_kernel_spmd(nc, in_maps, list(range(NCORES)))
    out = np.concatenate([r["out"] for r in res.results], axis=0)
    return out


# revision 41
# speedup vs baseline: 1.0027x; 1.0027x over previous
"""MAGC (multi-header attention global context) pooling kernel for Trainium2.

Math (per sample, reference.py):
    xh[g, n, :]   = x[n, g*64:(g+1)*64]                (g=8 headers, n=H*W)
    logits[g, n]  = (xh[g, n, :] . w_mask + b_mask) / 8
    attn          = softmax_n(logits)
    ctx[g, :]     = sum_n attn[g, n] * xh[g, n, :]     -> ctx [C]
    t             = relu(LN(ctx @ w1 + b1)) @ w2 + b2
    out           = x + t  (broadcast over n)

Sharding: pure data parallel, 16 samples -> 8 cores x 2 samples.

v2 design (per core, 2 samples):
  - x is kept in SBUF as f16 ONLY (xh, 60KB/partition/sample); the f32
    stream passes through a small staging pool and is converted by ACT.
    Both samples' xh fit resident -> full cross-sample pipelining.
  - DMA layout "(p k) c": each partition holds 4 consecutive HBM rows =
    one contiguous 8KB line (4x fewer descriptors than "(k p) c").
    Softmax/ctx are permutation-invariant over positions so relabeling
    is free; the store uses the same relabeling.
  - in-DMAs issue from the sync (SP) HWDGE queue, out-DMAs from the ACT
    HWDGE queue so the two streams' triggers don't head-of-line block.
  - residual add runs on gpsimd (f16+f16 -> f32 out staging); DVE does
    the logits mul+segmented reduce; ACT does converts+exp; PE does ctx.
    DVE(TT) and gpsimd both lock the shared SBUF port pair, but their
    combined locked time per chunk (~2.8us) fits in the DMA slot.
  - emission: A0 | B0 | interleave(C0, A1) | B1 | C1, where A=load+attn,
    B=softmax+MLP (serial), C=add+store. C0/A1 interleave chunk-wise so
    the out and in DMA streams share HBM bandwidth.
"""

import sys

import numpy as np

if "/opt/trn_rl_repo" not in sys.path:
    sys.path.insert(0, "/opt/trn_rl_repo")

B, H, W, C = 16, 48, 160, 512
G = 8                 # attention headers
SHI = C // G          # 64 channels per header
N = H * W             # 7680 spatial positions per sample
P = 128               # SBUF partitions
NT = N // P           # 60 [128, C] tiles per sample
NCORES = 8
BPC = B // NCORES     # samples per core
NB = C // P           # 4 channel blocks of 128
LN_EPS = 1e-3
KCH = 4               # [128, C] tiles per processing chunk (1 MB DMAs)
NCHK = NT // KCH      # 15 chunks per sample
OUT_DMA_ON_ACT = False  # issue store DMAs from the ACT HWDGE queue
PK_LAYOUT = True      # contiguous 8KB-per-partition DMA lines ("(p k) c")
INTERLEAVE = False     # interleave C0 with A1
REDUCE_ON_GP = False   # segmented logits reduce on gpsimd (frees DVE in A)
ADD_ON_DVE = True     # residual add on DVE (frees gpsimd in C)
CAST_DMA_OUT = False  # store via gpsimd casting DMA (f16->f32), no staging
PE_ADD = False        # residual add on PE via ident/ones matmuls into PSUM
PREFETCH = 2          # sample-1 chunks loaded before sample-0 MLP/store
EXP_LAG = True       # delay exp/ctx one chunk behind the load stream
MUL_ON_GP = False      # logits multiply on gpsimd (frees DVE in A)


def build_nc():
    import concourse.tile as tile
    from concourse import bacc, mybir
    from concourse.bass import broadcast_tensor_aps

    f32 = mybir.dt.float32
    f16 = mybir.dt.float16
    AX = mybir.AxisListType.X
    MUL = mybir.AluOpType.mult
    ADD = mybir.AluOpType.add
    SUB = mybir.AluOpType.subtract
    AF = mybir.ActivationFunctionType

    nc = bacc.Bacc()

    x_d = nc.dram_tensor("x", [BPC, H, W, C], f32, kind="ExternalInput")
    wrep_d = nc.dram_tensor("w_rep", [P, KCH, C], f16, kind="ExternalInput")
    bb_d = nc.dram_tensor("b_bias", [P, 1], f32, kind="ExternalInput")
    w1_d = nc.dram_tensor("w1p", [P, NB, C], f16, kind="ExternalInput")
    w2_d = nc.dram_tensor("w2p", [P, NB, C], f16, kind="ExternalInput")
    b1_d = nc.dram_tensor("b1r", [1, C], f16, kind="ExternalInput")
    b2_d = nc.dram_tensor("b2r", [1, C], f16, kind="ExternalInput")
    gm_d = nc.dram_tensor("gammar", [1, C], f16, kind="ExternalInput")
    bt_d = nc.dram_tensor("betar", [1, C], f16, kind="ExternalInput")
    ms_d = nc.dram_tensor("mask_sel", [G, C], f16, kind="ExternalInput")
    id_d = nc.dram_tensor("ident8", [G, G], f32, kind="ExternalInput")
    oh_d = nc.dram_tensor("ones_h", [1, P], f16, kind="ExternalInput")
    idh_d = nc.dram_tensor("identh", [P, P], f16, kind="ExternalInput")
    oc_d = nc.dram_tensor("ones_c", [P, 1], f32, kind="ExternalInput")
    out_d = nc.dram_tensor("out", [BPC, H, W, C], f32, kind="ExternalOutput")

    xf = x_d.rearrange("b h w c -> (b h w) c")
    of = out_d.rearrange("b h w c -> (b h w) c")

    with tile.TileContext(nc) as tc:
        with (
            tc.tile_pool(name="consts", bufs=1) as consts,
            tc.tile_pool(name="xhp", bufs=2) as xhp,
            tc.tile_pool(name="esbp", bufs=2) as esbp,
            tc.tile_pool(name="xinp", bufs=3) as xinp,
            tc.tile_pool(name="xwp", bufs=1) as xwp,
            tc.tile_pool(name="lgp", bufs=2) as lgp,
            tc.tile_pool(name="xoutp", bufs=3) as xoutp,
            tc.tile_pool(name="trp", bufs=1) as trp,
            tc.tile_pool(name="smp", bufs=1) as smp,
            tc.tile_pool(name="ctxps", bufs=1, space="PSUM") as ctxps,
            tc.tile_pool(name="mps", bufs=2, space="PSUM") as mps,
            tc.tile_pool(name="tps", bufs=1, space="PSUM") as tps,
            tc.tile_pool(name="addps", bufs=4, space="PSUM") as addps,
        ):
            w_rep = consts.tile([P, KCH, C], f16)
            nc.sync.dma_start(w_rep, wrep_d[:, :, :])
            bb = consts.tile([P, 1], f32)
            nc.sync.dma_start(bb, bb_d[:, :])
            eps_t = consts.tile([1, 1], f32)
            nc.vector.memset(eps_t, LN_EPS)

            def load_late_consts():
                """B-phase consts; DMAs queue behind A0's x loads so the
                first x chunk starts ~8us earlier."""
                w1s = consts.tile([P, NB, C], f16)
                nc.sync.dma_start(w1s, w1_d[:, :, :])
                w2s = consts.tile([P, NB, C], f16)
                nc.sync.dma_start(w2s, w2_d[:, :, :])
                b1s = consts.tile([1, C], f16)
                nc.sync.dma_start(b1s, b1_d[:, :])
                b2s = consts.tile([1, C], f16)
                nc.sync.dma_start(b2s, b2_d[:, :])
                gms = consts.tile([1, C], f16)
                nc.sync.dma_start(gms, gm_d[:, :])
                bts = consts.tile([1, C], f16)
                nc.sync.dma_start(bts, bt_d[:, :])
                msel = consts.tile([G, C], f16)
                nc.sync.dma_start(msel, ms_d[:, :])
                ident8 = consts.tile([G, G], f32)
                nc.sync.dma_start(ident8, id_d[:, :])
                ones_h = consts.tile([1, P], f16)
                nc.sync.dma_start(ones_h, oh_d[:, :])
                identh = consts.tile([P, P], f16)
                nc.sync.dma_start(identh, idh_d[:, :])
                ones_c = consts.tile([P, 1], f32)
                nc.sync.dma_start(ones_c, oc_d[:, :])
                return (w1s, w2s, b1s, b2s, gms, bts, msel, ident8,
                        ones_h, identh, ones_c)

            # Sem-absorption ops: walrus allows very few sync waits per
            # compute instruction, so let each engine observe the const-load
            # DMA sems via tiny reads up front, keeping hot-loop
            # instructions at <=1 wait each.
            ab_gp = smp.tile([1, 1], f32, tag="ab_gp")
            nc.gpsimd.tensor_copy(ab_gp, w_rep[0:1, 0, 0:1])
            ab_ac = smp.tile([1, 1], f32, tag="ab_ac")
            nc.scalar.copy(ab_ac, bb[0:1, 0:1])
            nc.scalar.copy(ab_ac, eps_t[0:1, 0:1])
            ab_dv = smp.tile([1, 1], f32, tag="ab_dv")
            nc.vector.tensor_copy(ab_dv, w_rep[0:1, 0, 0:1])

            def phase_a_load(s, ck, xh):
                """Load chunk ck of sample s, convert, mul, segmented reduce.

                Returns the logits tile; exp/ctx are emitted one chunk later
                (phase_a_tail) so the ACT queue's exp never stalls the next
                chunk's convert behind the DVE reduce."""
                base = s * N
                t0 = ck * KCH
                xin = xinp.tile([P, KCH, C], f32, tag="xin")
                rows = xf[base + t0 * P : base + (t0 + KCH) * P, :]
                nc.sync.dma_start(xin, rows.rearrange(IN_PAT, **IN_KW))
                nc.scalar.copy(xh[:, t0 : t0 + KCH, :], xin)
                xw = xwp.tile([P, KCH, C], f16, tag="xw")
                MUL_ENGINE.tensor_mul(xw, xh[:, t0 : t0 + KCH, :], w_rep)
                lg = lgp.tile([P, KCH, G], f16, tag="lg")
                with nc.allow_low_precision(
                    reason="64-term f16 logit sums; |logits|<1, exp next"
                ):
                    REDUCE_ENGINE.reduce_sum(
                        lg, xw.rearrange("p k (g s) -> p k g s", s=SHI), AX
                    )
                return lg

            def phase_a_tail(s, ck, lg, xh, esb, ctx_ps):
                t0 = ck * KCH
                # E = exp((dot + b_mask) / 8); |logits| < ~1 so no
                # max-subtraction is needed for stability.
                nc.scalar.activation(
                    esb[:, t0 : t0 + KCH, :], lg, AF.Exp,
                    bias=bb[:, 0:1], scale=0.125,
                )
                for t in range(t0, t0 + KCH):
                    # fp16 single-pass PE matmul: ctx[g, c] += sum_p E * xh
                    nc.tensor.matmul(
                        ctx_ps,
                        esb[:, t, :],
                        xh[:, t, :],
                        start=(t == 0),
                        stop=(t == NT - 1),
                    )

            def phase_a_chunk(s, ck, xh, esb, ctx_ps, pend):
                lg = phase_a_load(s, ck, xh)
                pend.append((ck, lg))
                if len(pend) > (1 if EXP_LAG else 0):
                    cka, lga = pend.pop(0)
                    phase_a_tail(s, cka, lga, xh, esb, ctx_ps)

            def phase_a_flush(s, xh, esb, ctx_ps, pend):
                while pend:
                    cka, lga = pend.pop(0)
                    phase_a_tail(s, cka, lga, xh, esb, ctx_ps)

            late = {}

            def phase_b(s, esb, ctx_ps):
                (w1s, w2s, b1s, b2s, gms, bts, msel, ident8,
                 ones_h, identh, ones_c) = late["c"]
                """Softmax denominator + MLP; returns trep [P, KCH, C] f16."""
                # S[g] = sum_{p,t} E[p, t, g]
                sp = lgp.tile([P, G], f32, tag="lg")
                nc.vector.reduce_sum(sp, esb.rearrange("p t g -> p g t"), AX)
                s_ps = mps.tile([G, 1], f32, tag="m")
                nc.tensor.matmul(s_ps, sp, ones_c, start=True, stop=True)
                sinv = smp.tile([G, 1], f32, tag="sinv")
                nc.vector.reciprocal(sinv, s_ps)

                # ctx extract: scale rows by 1/S, mask to the diagonal
                # header blocks, transpose to channel-major [128, 4]
                ctx_sm = smp.tile([G, C], f32, tag="ctx_sm")
                nc.vector.scalar_tensor_tensor(
                    out=ctx_sm, in0=ctx_ps, scalar=sinv, in1=msel,
                    op0=MUL, op1=MUL,
                )
                tp_list = []
                for j in range(NB):
                    tp = mps.tile([P, G], f32, tag="m")
                    nc.tensor.transpose(
                        tp, ctx_sm[:, j * P : (j + 1) * P], ident8
                    )
                    tp_list.append(tp)
                ctxt = smp.tile([P, NB], f16, tag="ctxt")
                with nc.allow_low_precision(
                    reason="8-term masked sum; f16 ctx feeds f16 matmul"
                ):
                    for j in range(NB):
                        nc.vector.reduce_sum(
                            ctxt[:, j : j + 1], tp_list[j], AX
                        )

                # h = ctx @ w1 + b1
                h_ps = mps.tile([1, C], f32, tag="m")
                for j in range(NB):
                    nc.tensor.matmul(
                        h_ps, ctxt[:, j : j + 1], w1s[:, j, :],
                        start=(j == 0), stop=False,
                    )
                nc.tensor.matmul(
                    h_ps, ones_h[:, 0:1], b1s, start=False, stop=True
                )

                # LayerNorm over C, then ReLU (all on DVE except one sqrt)
                musum = smp.tile([1, 1], f32, tag="musum")
                nc.vector.reduce_sum(musum, h_ps, AX)
                mu = smp.tile([1, 1], f32, tag="mu")
                nc.vector.tensor_scalar_mul(mu, musum, 1.0 / C)
                hc = smp.tile([1, C], f32, tag="hc")
                nc.vector.tensor_scalar(
                    out=hc, in0=h_ps, scalar1=mu, scalar2=None, op0=SUB
                )
                sq = smp.tile([1, C], f32, tag="sq")
                varsum = smp.tile([1, 1], f32, tag="varsum")
                nc.scalar.activation(sq, hc, AF.Square, accum_out=varsum)
                std = smp.tile([1, 1], f32, tag="std")
                nc.scalar.activation(
                    std, varsum, AF.Sqrt, bias=eps_t[:, 0:1], scale=1.0 / C
                )
                rstd = smp.tile([1, 1], f32, tag="rstd")
                nc.vector.reciprocal(rstd, std)
                hn = smp.tile([1, C], f32, tag="sq")
                nc.vector.scalar_tensor_tensor(
                    out=hn, in0=hc, scalar=rstd, in1=gms, op0=MUL, op1=MUL
                )
                hb = smp.tile([1, C], f32, tag="hc")
                nc.vector.tensor_add(hb, hn, bts)
                rl = smp.tile([1, C], f32, tag="rl")
                nc.vector.tensor_scalar_max(rl, hb, 0.0)

                # t = relu_h @ w2 + b2 (transpose relu_h to [128, 4] first)
                rt_ps = mps.tile([P, NB], f32, tag="m")
                for j in range(NB):
                    nc.tensor.transpose(
                        rt_ps[:, j : j + 1],
                        rl[:, j * P : (j + 1) * P],
                        ones_c[0:1, 0:1],
                    )
                rts = smp.tile([P, NB], f16, tag="rts")
                nc.vector.tensor_copy(rts, rt_ps)
                t_ps = mps.tile([1, C], f32, tag="m")
                for j in range(NB):
                    nc.tensor.matmul(
                        t_ps, rts[:, j : j + 1], w2s[:, j, :],
                        start=(j == 0), stop=False,
                    )
                nc.tensor.matmul(
                    t_ps, ones_h[:, 0:1], b2s, start=False, stop=True
                )
                tsb = smp.tile([1, C], f16, tag="tsb")
                nc.vector.tensor_copy(tsb, t_ps)

                if PE_ADD:
                    return tsb
                trep_ps = tps.tile([P, C], f32, tag="trep")
                nc.tensor.matmul(trep_ps, ones_h, tsb, start=True, stop=True)
                trep = trp.tile([P, KCH, C], f16, tag="trep_sb")
                for k in range(KCH):
                    nc.vector.tensor_copy(trep[:, k, :], trep_ps)
                return trep

            OUT_DMA_ENGINE = (
                nc.scalar.dma_start if OUT_DMA_ON_ACT else nc.sync.dma_start
            )
            REDUCE_ENGINE = nc.gpsimd if REDUCE_ON_GP else nc.vector
            MUL_ENGINE = nc.gpsimd if MUL_ON_GP else nc.vector
            ADD_ENGINE = nc.vector if ADD_ON_DVE else nc.gpsimd
            if PK_LAYOUT:
                IN_PAT, IN_KW = "(p k) c -> p k c", {"p": P}
            else:
                IN_PAT, IN_KW = "(k p) c -> p k c", {"p": P}

            def phase_c_chunk(s, ck, xh, trep):
                """Residual add (gpsimd) + store (ACT HWDGE queue)."""
                base = s * N
                t0 = ck * KCH
                if PE_ADD:
                    for h in range(2):
                        tt = t0 + 2 * h
                        add_ps = addps.tile([P, 2 * C], f32, tag="add")
                        nc.tensor.matmul(
                            add_ps,
                            identh,
                            xh[:, tt : tt + 2, :].reshape(P, 2 * C),
                            start=True, stop=False,
                        )
                        nc.tensor.matmul(
                            add_ps, ones_h, trep, start=False, stop=True
                        )
                        xout = xoutp.tile([P, 2 * C], f32, tag="xout")
                        nc.scalar.copy(xout, add_ps)
                        rows = of[
                            base + tt * P : base + (tt + 2) * P, :
                        ]
                        OUT_DMA_ENGINE(
                            rows.rearrange(IN_PAT, **IN_KW).reshape(
                                P, 2 * C
                            ),
                            xout,
                        )
                    return
                rows = of[base + t0 * P : base + (t0 + KCH) * P, :]
                if CAST_DMA_OUT:
                    xadd = xwp.tile([P, KCH, C], f16, tag="xw")
                    with nc.allow_low_precision(
                        reason="residual add in f16; |out|<8, gate 2e-2"
                    ):
                        ADD_ENGINE.tensor_add(
                            xadd, xh[:, t0 : t0 + KCH, :], trep
                        )
                    # SWDGE casting DMA: f16 SBUF -> f32 HBM in flight
                    nc.gpsimd.dma_start(
                        rows.rearrange(IN_PAT, **IN_KW), xadd
                    )
                else:
                    xout = xoutp.tile([P, KCH, C], f32, tag="xout")
                    ADD_ENGINE.tensor_add(
                        xout, xh[:, t0 : t0 + KCH, :], trep
                    )
                    OUT_DMA_ENGINE(rows.rearrange(IN_PAT, **IN_KW), xout)

            # ---- sample 0: load + attention
            xh0 = xhp.tile([P, NT, C], f16, tag="xh")
            esb0 = esbp.tile([P, NT, G], f16, tag="esb")
            ctx0 = ctxps.tile([G, C], f32, tag="ctx")
            pend0 = []
            for ck in range(3):
                phase_a_chunk(0, ck, xh0, esb0, ctx0, pend0)
            late["c"] = load_late_consts()
            for ck in range(3, NCHK):
                phase_a_chunk(0, ck, xh0, esb0, ctx0, pend0)
            phase_a_flush(0, xh0, esb0, ctx0, pend0)
            # ---- prefetch first sample-1 chunks (their DMAs overlap B0)
            xh1 = xhp.tile([P, NT, C], f16, tag="xh")
            esb1 = esbp.tile([P, NT, G], f16, tag="esb")
            ctx1 = ctxps.tile([G, C], f32, tag="ctx")
            pend1 = []
            for ck in range(PREFETCH):
                phase_a_chunk(1, ck, xh1, esb1, ctx1, pend1)
            # ---- sample 0 MLP
            trep0 = phase_b(0, esb0, ctx0)
            # ---- interleave: store sample 0 / load+attn sample 1
            if INTERLEAVE:
                for ck in range(NCHK):
                    phase_c_chunk(0, ck, xh0, trep0)
                    if PREFETCH + ck < NCHK:
                        phase_a_chunk(
                            1, PREFETCH + ck, xh1, esb1, ctx1, pend1
                        )
            else:
                for ck in range(PREFETCH, NCHK):
                    phase_a_chunk(1, ck, xh1, esb1, ctx1, pend1)
                for ck in range(NCHK):
                    phase_c_chunk(0, ck, xh0, trep0)
            phase_a_flush(1, xh1, esb1, ctx1, pend1)
            # ---- sample 1 MLP + store
            trep1 = phase_b(1, esb1, ctx1)
            for ck in range(NCHK):
                phase_c_chunk(1, ck, xh1, trep1)

    nc.finalize()
    return nc


def _prep_shared(inputs):
    w_mask = np.asarray(inputs["w_mask"], np.float32).reshape(SHI)
    b_mask = np.asarray(inputs["b_mask"], np.float32).reshape(1)
    w1 = np.asarray(inputs["w1"], np.float32)
    w2 = np.asarray(inputs["w2"], np.float32)

    shared = {
        "w_rep": np.broadcast_to(
            np.tile(w_mask, G), (P, KCH, C)
        ).astype(np.float16),
        "b_bias": np.full((P, 1), b_mask[0] * 0.125, np.float32),
        "w1p": np.ascontiguousarray(
            w1.reshape(NB, P, C).transpose(1, 0, 2)
        ).astype(np.float16),
        "w2p": np.ascontiguousarray(
            w2.reshape(NB, P, C).transpose(1, 0, 2)
        ).astype(np.float16),
        "b1r": np.asarray(inputs["b1"], np.float16).reshape(1, C),
        "b2r": np.asarray(inputs["b2"], np.float16).reshape(1, C),
        "gammar": np.asarray(inputs["gamma"], np.float16).reshape(1, C),
        "betar": np.asarray(inputs["beta"], np.float16).reshape(1, C),
        "mask_sel": (
            (np.arange(C)[None, :] // SHI) == np.arange(G)[:, None]
        ).astype(np.float16),
        "ident8": np.eye(G, dtype=np.float32),
        "ones_h": np.ones((1, P), np.float16),
        "identh": np.eye(P, dtype=np.float16),
        "ones_c": np.ones((P, 1), np.float32),
    }
    return shared


def make_in_maps(inputs):
    x = np.asarray(inputs["x"], np.float32)
    shared = _prep_shared(inputs)
    in_maps = []
    for i in range(NCORES):
        m = dict(shared)
        m["x"] = np.ascontiguousarray(x[i * BPC : (i + 1) * BPC])
        in_maps.append(m)
    return in_maps


def _axon_device_reset():
    """Clear any wedged NRT exec-unit state left by a previous session."""
    try:
        import ctypes

        import jax

        jax.devices()
        lib = ctypes.CDLL("/opt/axon/libaxon_pjrt.so")
        lib.axon_reset.restype = ctypes.c_int64
        lib.axon_reset()
    except Exception:
        pass


def kernel(**inputs):
    from concourse.bass_utils import run_bass_kernel_spmd

    _axon_device_reset()
    nc = build_nc()
    in_maps = make_in_maps(inputs)
    res = run_bass_kernel_spmd(nc, in_maps, list(range(NCORES)))
    out = np.concatenate([r["out"] for r in res.results], axis=0)
    return out


# revision 42
# speedup vs baseline: 1.0089x; 1.0062x over previous
"""MAGC (multi-header attention global context) pooling kernel for Trainium2.

Math (per sample, reference.py):
    xh[g, n, :]   = x[n, g*64:(g+1)*64]                (g=8 headers, n=H*W)
    logits[g, n]  = (xh[g, n, :] . w_mask + b_mask) / 8
    attn          = softmax_n(logits)
    ctx[g, :]     = sum_n attn[g, n] * xh[g, n, :]     -> ctx [C]
    t             = relu(LN(ctx @ w1 + b1)) @ w2 + b2
    out           = x + t  (broadcast over n)

Sharding: pure data parallel, 16 samples -> 8 cores x 2 samples.

v2 design (per core, 2 samples):
  - x is kept in SBUF as f16 ONLY (xh, 60KB/partition/sample); the f32
    stream passes through a small staging pool and is converted by ACT.
    Both samples' xh fit resident -> full cross-sample pipelining.
  - DMA layout "(p k) c": each partition holds 4 consecutive HBM rows =
    one contiguous 8KB line (4x fewer descriptors than "(k p) c").
    Softmax/ctx are permutation-invariant over positions so relabeling
    is free; the store uses the same relabeling.
  - in-DMAs issue from the sync (SP) HWDGE queue, out-DMAs from the ACT
    HWDGE queue so the two streams' triggers don't head-of-line block.
  - residual add runs on gpsimd (f16+f16 -> f32 out staging); DVE does
    the logits mul+segmented reduce; ACT does converts+exp; PE does ctx.
    DVE(TT) and gpsimd both lock the shared SBUF port pair, but their
    combined locked time per chunk (~2.8us) fits in the DMA slot.
  - emission: A0 | B0 | interleave(C0, A1) | B1 | C1, where A=load+attn,
    B=softmax+MLP (serial), C=add+store. C0/A1 interleave chunk-wise so
    the out and in DMA streams share HBM bandwidth.
"""

import sys

import numpy as np

if "/opt/trn_rl_repo" not in sys.path:
    sys.path.insert(0, "/opt/trn_rl_repo")

B, H, W, C = 16, 48, 160, 512
G = 8                 # attention headers
SHI = C // G          # 64 channels per header
N = H * W             # 7680 spatial positions per sample
P = 128               # SBUF partitions
NT = N // P           # 60 [128, C] tiles per sample
NCORES = 8
BPC = B // NCORES     # samples per core
NB = C // P           # 4 channel blocks of 128
LN_EPS = 1e-3
KCH = 4               # [128, C] tiles per processing chunk (1 MB DMAs)
NCHK = NT // KCH      # 15 chunks per sample
OUT_DMA_ON_ACT = False  # issue store DMAs from the ACT HWDGE queue
PK_LAYOUT = True      # contiguous 8KB-per-partition DMA lines ("(p k) c")
INTERLEAVE = False     # interleave C0 with A1
REDUCE_ON_GP = False   # segmented logits reduce on gpsimd (frees DVE in A)
ADD_ON_DVE = True     # residual add on DVE (frees gpsimd in C)
CAST_DMA_OUT = False  # store via gpsimd casting DMA (f16->f32), no staging
PE_ADD = False        # residual add on PE via ident/ones matmuls into PSUM
PREFETCH = 3          # sample-1 chunks loaded before sample-0 MLP/store
EXP_LAG = True       # delay exp/ctx one chunk behind the load stream
MUL_ON_GP = False      # logits multiply on gpsimd (frees DVE in A)


def build_nc():
    import concourse.tile as tile
    from concourse import bacc, mybir
    from concourse.bass import broadcast_tensor_aps

    f32 = mybir.dt.float32
    f16 = mybir.dt.float16
    AX = mybir.AxisListType.X
    MUL = mybir.AluOpType.mult
    ADD = mybir.AluOpType.add
    SUB = mybir.AluOpType.subtract
    AF = mybir.ActivationFunctionType

    nc = bacc.Bacc()

    x_d = nc.dram_tensor("x", [BPC, H, W, C], f32, kind="ExternalInput")
    wrep_d = nc.dram_tensor("w_rep", [P, KCH, C], f16, kind="ExternalInput")
    bb_d = nc.dram_tensor("b_bias", [P, 1], f32, kind="ExternalInput")
    w1_d = nc.dram_tensor("w1p", [P, NB, C], f16, kind="ExternalInput")
    w2_d = nc.dram_tensor("w2p", [P, NB, C], f16, kind="ExternalInput")
    b1_d = nc.dram_tensor("b1r", [1, C], f16, kind="ExternalInput")
    b2_d = nc.dram_tensor("b2r", [1, C], f16, kind="ExternalInput")
    gm_d = nc.dram_tensor("gammar", [1, C], f16, kind="ExternalInput")
    bt_d = nc.dram_tensor("betar", [1, C], f16, kind="ExternalInput")
    ms_d = nc.dram_tensor("mask_sel", [G, C], f16, kind="ExternalInput")
    id_d = nc.dram_tensor("ident8", [G, G], f32, kind="ExternalInput")
    oh_d = nc.dram_tensor("ones_h", [1, P], f16, kind="ExternalInput")
    idh_d = nc.dram_tensor("identh", [P, P], f16, kind="ExternalInput")
    oc_d = nc.dram_tensor("ones_c", [P, 1], f32, kind="ExternalInput")
    out_d = nc.dram_tensor("out", [BPC, H, W, C], f32, kind="ExternalOutput")

    xf = x_d.rearrange("b h w c -> (b h w) c")
    of = out_d.rearrange("b h w c -> (b h w) c")

    with tile.TileContext(nc) as tc:
        with (
            tc.tile_pool(name="consts", bufs=1) as consts,
            tc.tile_pool(name="xhp", bufs=2) as xhp,
            tc.tile_pool(name="esbp", bufs=2) as esbp,
            tc.tile_pool(name="xinp", bufs=3) as xinp,
            tc.tile_pool(name="xwp", bufs=1) as xwp,
            tc.tile_pool(name="lgp", bufs=2) as lgp,
            tc.tile_pool(name="xoutp", bufs=3) as xoutp,
            tc.tile_pool(name="trp", bufs=1) as trp,
            tc.tile_pool(name="smp", bufs=1) as smp,
            tc.tile_pool(name="ctxps", bufs=1, space="PSUM") as ctxps,
            tc.tile_pool(name="mps", bufs=2, space="PSUM") as mps,
            tc.tile_pool(name="tps", bufs=1, space="PSUM") as tps,
            tc.tile_pool(name="addps", bufs=4, space="PSUM") as addps,
        ):
            w_rep = consts.tile([P, KCH, C], f16)
            nc.sync.dma_start(w_rep, wrep_d[:, :, :])
            bb = consts.tile([P, 1], f32)
            nc.sync.dma_start(bb, bb_d[:, :])
            w1s = consts.tile([P, NB, C], f16)
            nc.sync.dma_start(w1s, w1_d[:, :, :])
            w2s = consts.tile([P, NB, C], f16)
            nc.sync.dma_start(w2s, w2_d[:, :, :])
            b1s = consts.tile([1, C], f16)
            nc.sync.dma_start(b1s, b1_d[:, :])
            b2s = consts.tile([1, C], f16)
            nc.sync.dma_start(b2s, b2_d[:, :])
            gms = consts.tile([1, C], f16)
            nc.sync.dma_start(gms, gm_d[:, :])
            bts = consts.tile([1, C], f16)
            nc.sync.dma_start(bts, bt_d[:, :])
            msel = consts.tile([G, C], f16)
            nc.sync.dma_start(msel, ms_d[:, :])
            ident8 = consts.tile([G, G], f32)
            nc.sync.dma_start(ident8, id_d[:, :])
            ones_h = consts.tile([1, P], f16)
            nc.sync.dma_start(ones_h, oh_d[:, :])
            identh = consts.tile([P, P], f16)
            nc.sync.dma_start(identh, idh_d[:, :])
            ones_c = consts.tile([P, 1], f32)
            nc.sync.dma_start(ones_c, oc_d[:, :])
            eps_t = consts.tile([1, 1], f32)
            nc.vector.memset(eps_t, LN_EPS)

            # Sem-absorption ops: walrus allows very few sync waits per
            # compute instruction, so let each engine observe the const-load
            # DMA sems via tiny reads up front, keeping hot-loop
            # instructions at <=1 wait each.
            ab_gp = smp.tile([1, 1], f32, tag="ab_gp")
            nc.gpsimd.tensor_copy(ab_gp, w_rep[0:1, 0, 0:1])
            ab_ac = smp.tile([1, 1], f32, tag="ab_ac")
            nc.scalar.copy(ab_ac, bb[0:1, 0:1])
            nc.scalar.copy(ab_ac, eps_t[0:1, 0:1])
            ab_dv = smp.tile([1, 1], f32, tag="ab_dv")
            nc.vector.tensor_copy(ab_dv, msel[0:1, 0:1])

            def phase_a_load(s, ck, xh):
                """Load chunk ck of sample s, convert, mul, segmented reduce.

                Returns the logits tile; exp/ctx are emitted one chunk later
                (phase_a_tail) so the ACT queue's exp never stalls the next
                chunk's convert behind the DVE reduce."""
                base = s * N
                t0 = ck * KCH
                xin = xinp.tile([P, KCH, C], f32, tag="xin")
                rows = xf[base + t0 * P : base + (t0 + KCH) * P, :]
                nc.sync.dma_start(xin, rows.rearrange(IN_PAT, **IN_KW))
                nc.scalar.copy(xh[:, t0 : t0 + KCH, :], xin)
                xw = xwp.tile([P, KCH, C], f16, tag="xw")
                MUL_ENGINE.tensor_mul(xw, xh[:, t0 : t0 + KCH, :], w_rep)
                lg = lgp.tile([P, KCH, G], f16, tag="lg")
                with nc.allow_low_precision(
                    reason="64-term f16 logit sums; |logits|<1, exp next"
                ):
                    REDUCE_ENGINE.reduce_sum(
                        lg, xw.rearrange("p k (g s) -> p k g s", s=SHI), AX
                    )
                return lg

            def phase_a_tail(s, ck, lg, xh, esb, ctx_ps):
                t0 = ck * KCH
                # E = exp((dot + b_mask) / 8); |logits| < ~1 so no
                # max-subtraction is needed for stability.
                nc.scalar.activation(
                    esb[:, t0 : t0 + KCH, :], lg, AF.Exp,
                    bias=bb[:, 0:1], scale=0.125,
                )
                for t in range(t0, t0 + KCH):
                    # fp16 single-pass PE matmul: ctx[g, c] += sum_p E * xh
                    nc.tensor.matmul(
                        ctx_ps,
                        esb[:, t, :],
                        xh[:, t, :],
                        start=(t == 0),
                        stop=(t == NT - 1),
                    )

            def phase_a_chunk(s, ck, xh, esb, ctx_ps, pend):
                lg = phase_a_load(s, ck, xh)
                pend.append((ck, lg))
                if len(pend) > (1 if EXP_LAG else 0):
                    cka, lga = pend.pop(0)
                    phase_a_tail(s, cka, lga, xh, esb, ctx_ps)

            def phase_a_flush(s, xh, esb, ctx_ps, pend):
                while pend:
                    cka, lga = pend.pop(0)
                    phase_a_tail(s, cka, lga, xh, esb, ctx_ps)

            def phase_b(s, esb, ctx_ps):
                """Softmax denominator + MLP; returns trep [P, KCH, C] f16."""
                # S[g] = sum_{p,t} E[p, t, g]
                sp = lgp.tile([P, G], f32, tag="lg")
                nc.vector.reduce_sum(sp, esb.rearrange("p t g -> p g t"), AX)
                s_ps = mps.tile([G, 1], f32, tag="m")
                nc.tensor.matmul(s_ps, sp, ones_c, start=True, stop=True)
                sinv = smp.tile([G, 1], f32, tag="sinv")
                nc.vector.reciprocal(sinv, s_ps)

                # ctx extract: scale rows by 1/S, mask to the diagonal
                # header blocks, transpose to channel-major [128, 4]
                ctx_sm = smp.tile([G, C], f32, tag="ctx_sm")
                nc.vector.scalar_tensor_tensor(
                    out=ctx_sm, in0=ctx_ps, scalar=sinv, in1=msel,
                    op0=MUL, op1=MUL,
                )
                tp_list = []
                for j in range(NB):
                    tp = mps.tile([P, G], f32, tag="m")
                    nc.tensor.transpose(
                        tp, ctx_sm[:, j * P : (j + 1) * P], ident8
                    )
                    tp_list.append(tp)
                ctxt = smp.tile([P, NB], f16, tag="ctxt")
                with nc.allow_low_precision(
                    reason="8-term masked sum; f16 ctx feeds f16 matmul"
                ):
                    for j in range(NB):
                        nc.vector.reduce_sum(
                            ctxt[:, j : j + 1], tp_list[j], AX
                        )

                # h = ctx @ w1 + b1
                h_ps = mps.tile([1, C], f32, tag="m")
                for j in range(NB):
                    nc.tensor.matmul(
                        h_ps, ctxt[:, j : j + 1], w1s[:, j, :],
                        start=(j == 0), stop=False,
                    )
                nc.tensor.matmul(
                    h_ps, ones_h[:, 0:1], b1s, start=False, stop=True
                )

                # LayerNorm over C, then ReLU (all on DVE except one sqrt)
                musum = smp.tile([1, 1], f32, tag="musum")
                nc.vector.reduce_sum(musum, h_ps, AX)
                mu = smp.tile([1, 1], f32, tag="mu")
                nc.vector.tensor_scalar_mul(mu, musum, 1.0 / C)
                hc = smp.tile([1, C], f32, tag="hc")
                nc.vector.tensor_scalar(
                    out=hc, in0=h_ps, scalar1=mu, scalar2=None, op0=SUB
                )
                sq = smp.tile([1, C], f32, tag="sq")
                varsum = smp.tile([1, 1], f32, tag="varsum")
                nc.scalar.activation(sq, hc, AF.Square, accum_out=varsum)
                std = smp.tile([1, 1], f32, tag="std")
                nc.scalar.activation(
                    std, varsum, AF.Sqrt, bias=eps_t[:, 0:1], scale=1.0 / C
                )
                rstd = smp.tile([1, 1], f32, tag="rstd")
                nc.vector.reciprocal(rstd, std)
                hn = smp.tile([1, C], f32, tag="sq")
                nc.vector.scalar_tensor_tensor(
                    out=hn, in0=hc, scalar=rstd, in1=gms, op0=MUL, op1=MUL
                )
                hb = smp.tile([1, C], f32, tag="hc")
                nc.vector.tensor_add(hb, hn, bts)
                rl = smp.tile([1, C], f32, tag="rl")
                nc.vector.tensor_scalar_max(rl, hb, 0.0)

                # t = relu_h @ w2 + b2 (transpose relu_h to [128, 4] first)
                rt_ps = mps.tile([P, NB], f32, tag="m")
                for j in range(NB):
                    nc.tensor.transpose(
                        rt_ps[:, j : j + 1],
                        rl[:, j * P : (j + 1) * P],
                        ones_c[0:1, 0:1],
                    )
                rts = smp.tile([P, NB], f16, tag="rts")
                nc.vector.tensor_copy(rts, rt_ps)
                t_ps = mps.tile([1, C], f32, tag="m")
                for j in range(NB):
                    nc.tensor.matmul(
                        t_ps, rts[:, j : j + 1], w2s[:, j, :],
                        start=(j == 0), stop=False,
                    )
                nc.tensor.matmul(
                    t_ps, ones_h[:, 0:1], b2s, start=False, stop=True
                )
                tsb = smp.tile([1, C], f16, tag="tsb")
                nc.vector.tensor_copy(tsb, t_ps)

                if PE_ADD:
                    return tsb
                trep_ps = tps.tile([P, C], f32, tag="trep")
                nc.tensor.matmul(trep_ps, ones_h, tsb, start=True, stop=True)
                trep = trp.tile([P, KCH, C], f16, tag="trep_sb")
                for k in range(KCH):
                    nc.vector.tensor_copy(trep[:, k, :], trep_ps)
                return trep

            OUT_DMA_ENGINE = (
                nc.scalar.dma_start if OUT_DMA_ON_ACT else nc.sync.dma_start
            )
            REDUCE_ENGINE = nc.gpsimd if REDUCE_ON_GP else nc.vector
            MUL_ENGINE = nc.gpsimd if MUL_ON_GP else nc.vector
            ADD_ENGINE = nc.vector if ADD_ON_DVE else nc.gpsimd
            if PK_LAYOUT:
                IN_PAT, IN_KW = "(p k) c -> p k c", {"p": P}
            else:
                IN_PAT, IN_KW = "(k p) c -> p k c", {"p": P}

            def phase_c_chunk(s, ck, xh, trep):
                """Residual add (gpsimd) + store (ACT HWDGE queue)."""
                base = s * N
                t0 = ck * KCH
                if PE_ADD:
                    for h in range(2):
                        tt = t0 + 2 * h
                        add_ps = addps.tile([P, 2 * C], f32, tag="add")
                        nc.tensor.matmul(
                            add_ps,
                            identh,
                            xh[:, tt : tt + 2, :].reshape(P, 2 * C),
                            start=True, stop=False,
                        )
                        nc.tensor.matmul(
                            add_ps, ones_h, trep, start=False, stop=True
                        )
                        xout = xoutp.tile([P, 2 * C], f32, tag="xout")
                        nc.scalar.copy(xout, add_ps)
                        rows = of[
                            base + tt * P : base + (tt + 2) * P, :
                        ]
                        OUT_DMA_ENGINE(
                            rows.rearrange(IN_PAT, **IN_KW).reshape(
                                P, 2 * C
                            ),
                            xout,
                        )
                    return
                rows = of[base + t0 * P : base + (t0 + KCH) * P, :]
                if CAST_DMA_OUT:
                    xadd = xwp.tile([P, KCH, C], f16, tag="xw")
                    with nc.allow_low_precision(
                        reason="residual add in f16; |out|<8, gate 2e-2"
                    ):
                        ADD_ENGINE.tensor_add(
                            xadd, xh[:, t0 : t0 + KCH, :], trep
                        )
                    # SWDGE casting DMA: f16 SBUF -> f32 HBM in flight
                    nc.gpsimd.dma_start(
                        rows.rearrange(IN_PAT, **IN_KW), xadd
                    )
                else:
                    xout = xoutp.tile([P, KCH, C], f32, tag="xout")
                    ADD_ENGINE.tensor_add(
                        xout, xh[:, t0 : t0 + KCH, :], trep
                    )
                    OUT_DMA_ENGINE(rows.rearrange(IN_PAT, **IN_KW), xout)

            # ---- sample 0: load + attention
            xh0 = xhp.tile([P, NT, C], f16, tag="xh")
            esb0 = esbp.tile([P, NT, G], f16, tag="esb")
            ctx0 = ctxps.tile([G, C], f32, tag="ctx")
            pend0 = []
            for ck in range(NCHK):
                phase_a_chunk(0, ck, xh0, esb0, ctx0, pend0)
            phase_a_flush(0, xh0, esb0, ctx0, pend0)
            # ---- prefetch first sample-1 chunks (their DMAs overlap B0)
            xh1 = xhp.tile([P, NT, C], f16, tag="xh")
            esb1 = esbp.tile([P, NT, G], f16, tag="esb")
            ctx1 = ctxps.tile([G, C], f32, tag="ctx")
            pend1 = []
            for ck in range(PREFETCH):
                phase_a_chunk(1, ck, xh1, esb1, ctx1, pend1)
            # ---- sample 0 MLP
            trep0 = phase_b(0, esb0, ctx0)
            # ---- interleave: store sample 0 / load+attn sample 1
            if INTERLEAVE:
                for ck in range(NCHK):
                    phase_c_chunk(0, ck, xh0, trep0)
                    if PREFETCH + ck < NCHK:
                        phase_a_chunk(
                            1, PREFETCH + ck, xh1, esb1, ctx1, pend1
                        )
            else:
                for ck in range(PREFETCH, NCHK):
                    phase_a_chunk(1, ck, xh1, esb1, ctx1, pend1)
                for ck in range(NCHK):
                    phase_c_chunk(0, ck, xh0, trep0)
            phase_a_flush(1, xh1, esb1, ctx1, pend1)
            # ---- sample 1 MLP + store
            trep1 = phase_b(1, esb1, ctx1)
            for ck in range(NCHK):
                phase_c_chunk(1, ck, xh1, trep1)

    nc.finalize()
    return nc


def _prep_shared(inputs):
    w_mask = np.asarray(inputs["w_mask"], np.float32).reshape(SHI)
    b_mask = np.asarray(inputs["b_mask"], np.float32).reshape(1)
    w1 = np.asarray(inputs["w1"], np.float32)
    w2 = np.asarray(inputs["w2"], np.float32)

    shared = {
        "w_rep": np.broadcast_to(
            np.tile(w_mask, G), (P, KCH, C)
        ).astype(np.float16),
        "b_bias": np.full((P, 1), b_mask[0] * 0.125, np.float32),
        "w1p": np.ascontiguousarray(
            w1.reshape(NB, P, C).transpose(1, 0, 2)
        ).astype(np.float16),
        "w2p": np.ascontiguousarray(
            w2.reshape(NB, P, C).transpose(1, 0, 2)
        ).astype(np.float16),
        "b1r": np.asarray(inputs["b1"], np.float16).reshape(1, C),
        "b2r": np.asarray(inputs["b2"], np.float16).reshape(1, C),
        "gammar": np.asarray(inputs["gamma"], np.float16).reshape(1, C),
        "betar": np.asarray(inputs["beta"], np.float16).reshape(1, C),
        "mask_sel": (
            (np.arange(C)[None, :] // SHI) == np.arange(G)[:, None]
        ).astype(np.float16),
        "ident8": np.eye(G, dtype=np.float32),
        "ones_h": np.ones((1, P), np.float16),
        "identh": np.eye(P, dtype=np.float16),
        "ones_c": np.ones((P, 1), np.float32),
    }
    return shared


def make_in_maps(inputs):
    x = np.asarray(inputs["x"], np.float32)
    shared = _prep_shared(inputs)
    in_maps = []
    for i in range(NCORES):
        m = dict(shared)
        m["x"] = np.ascontiguousarray(x[i * BPC : (i + 1) * BPC])
        in_maps.append(m)
    return in_maps


def _axon_device_reset():
    """Clear any wedged NRT exec-unit state left by a previous session."""
    try:
        import ctypes

        import jax

        jax.devices()
        lib = ctypes.CDLL("/opt/axon/libaxon_pjrt.so")
        lib.axon_reset.restype = ctypes.c_int64
        lib.axon_reset()
    except Exception:
        pass


def kernel(**inputs):
    from concourse.bass_utils import run_bass_kernel_spmd

    _axon_device_reset()
    nc = build_nc()
    in_maps = make_in_maps(inputs)
    res = run_bass_kernel_spmd(nc, in_maps, list(range(NCORES)))
    out = np.concatenate([r["out"] for r in res.results], axis=0)
    return out
